# revision 8
# baseline (speedup 1.0000x reference)
"""Trainium2 Bass kernel for a single-query-head attention block with KV-cache
update (B=16, S=8, D=2048, H=16, HD=128, MAXSEQ=4096), data-parallel over
batch across 8 NeuronCores (2 batches per core).

kernel(**inputs) takes the full unsharded numpy inputs and returns
(out, key_all, val_all) matching the reference.

Algorithm notes: seq_pos rows are contiguous (base..base+S-1), so the causal
mask over the updated cache reduces to: all original cache rows < base are
fully visible, rows >= base are replaced/masked, and the S new tokens attend
causally among themselves. The old-row mask is folded into the softmax exp()
as a per-partition additive bias (-30000 => exp underflows to 0). Masked
positions contribute < MAXSEQ*e^-50 relative to the softmax sum, far below
f32 resolution, so dropping them matches the reference within tolerance.
The updated caches are assembled on the host (8 contiguous rows per batch);
the device computes the attention output and the new k/v rows.
"""
import sys

sys.path.insert(0, "/opt/trn_rl_repo")

import numpy as np

import concourse.bacc as bacc
import concourse.bass as bass
import concourse.mybir as mybir
from concourse import tile
from concourse import bass_utils

B, S, D, H, HD, MAXSEQ = 16, 8, 2048, 16, 128, 4096
WAVELENGTH = 10000.0
SOFTCAP = 50.0
EPS = 1e-6
NCORES = 8
BPC = B // NCORES          # batches per core = 2
T = BPC * S                # tokens per core = 16
NCH = MAXSEQ // 128        # cache chunks per batch = 32
MASKBIAS = -30000.0
F32 = mybir.dt.float32

_CACHE = {}


def _emit(nc):
    """Emit the per-core program. All 8 cores run this same program (SPMD)."""
    dt = F32
    # ---- DRAM I/O ----
    xt_d = nc.dram_tensor("xt", [128, 16, T], dt, kind="ExternalInput")      # x^T: [p, chunk, tok]
    ck_d = nc.dram_tensor("ck", [BPC, MAXSEQ, H * HD], dt, kind="ExternalInput")
    cv_d = nc.dram_tensor("cv", [BPC, MAXSEQ, H * HD], dt, kind="ExternalInput")
    wq_d = nc.dram_tensor("wq", [D, HD], dt, kind="ExternalInput")
    wk_d = nc.dram_tensor("wk", [D, H * HD], dt, kind="ExternalInput")
    wv_d = nc.dram_tensor("wv", [D, H * HD], dt, kind="ExternalInput")
    wo_d = nc.dram_tensor("wo", [H * HD, D], dt, kind="ExternalInput")
    bq_d = nc.dram_tensor("bq", [1, HD], dt, kind="ExternalInput")
    bk_d = nc.dram_tensor("bk", [1, H * HD], dt, kind="ExternalInput")
    bv_d = nc.dram_tensor("bv", [1, H * HD], dt, kind="ExternalInput")
    bo_d = nc.dram_tensor("bo", [1, D], dt, kind="ExternalInput")
    qse_d = nc.dram_tensor("qse", [1, HD], dt, kind="ExternalInput")         # q_scale/sqrt(HD)
    kse_d = nc.dram_tensor("kse", [1, HD], dt, kind="ExternalInput")
    cosq_d = nc.dram_tensor("cosq", [T, HD], dt, kind="ExternalInput")
    ssq_d = nc.dram_tensor("ssq", [T, HD], dt, kind="ExternalInput")         # [-sin, +sin]
    cosk_d = nc.dram_tensor("cosk", [T, H * HD], dt, kind="ExternalInput")
    ssk_d = nc.dram_tensor("ssk", [T, H * HD], dt, kind="ExternalInput")
    tri_d = nc.dram_tensor("tri", [S, H * S], dt, kind="ExternalInput")      # causal 0/1
    mb_d = nc.dram_tensor("mb", [128, BPC * NCH], dt, kind="ExternalInput")  # mask bias cols
    id_d = nc.dram_tensor("ident", [128, 128], dt, kind="ExternalInput")
    ones_d = nc.dram_tensor("ones", [128, 1], dt, kind="ExternalInput")
    onesr_d = nc.dram_tensor("onesr", [1, H * S], dt, kind="ExternalInput")
    out_d = nc.dram_tensor("out", [T, D], dt, kind="ExternalOutput")
    ko_d = nc.dram_tensor("ko", [T, H * HD], dt, kind="ExternalOutput")
    vo_d = nc.dram_tensor("vo", [T, H * HD], dt, kind="ExternalOutput")

    tanh = mybir.ActivationFunctionType.Tanh
    expf = mybir.ActivationFunctionType.Exp
    sqrtf = mybir.ActivationFunctionType.Sqrt
    mult = mybir.AluOpType.mult
    addop = mybir.AluOpType.add

    with tile.TileContext(nc) as tc:
        with (
            tc.tile_pool(name="const", bufs=1) as cp,
            tc.tile_pool(name="wp", bufs=4) as wp,
            tc.tile_pool(name="kvp", bufs=3) as kvp,
            tc.tile_pool(name="ktrp", bufs=3) as ktrp,
            tc.tile_pool(name="actp", bufs=3) as actp,
            tc.tile_pool(name="work", bufs=1) as work,
            tc.tile_pool(name="pstp", bufs=2, space="PSUM") as pstp,
            tc.tile_pool(name="pssp", bufs=2, space="PSUM") as pssp,
            tc.tile_pool(name="pprojp", bufs=2, space="PSUM") as pprojp,
            tc.tile_pool(name="pxp", bufs=1, space="PSUM") as pxp,
            tc.tile_pool(name="pdenp", bufs=1, space="PSUM") as pdenp,
        ):
            # ---- constants into SBUF ----
            ident = cp.tile([128, 128], dt)
            nc.sync.dma_start(ident, id_d[:])
            ones = cp.tile([128, 1], dt)
            nc.sync.dma_start(ones, ones_d[:])
            tri = cp.tile([S, H * S], dt)
            nc.sync.dma_start(tri, tri_d[:])
            mb = cp.tile([128, BPC * NCH], dt)
            nc.sync.dma_start(mb, mb_d[:])
            xt = cp.tile([128, 16, T], dt)
            nc.sync.dma_start(xt, xt_d[:])
            bq = cp.tile([T, HD], dt)
            nc.sync.dma_start(bq, bq_d[:].to_broadcast((T, HD)))
            bk = cp.tile([T, H * HD], dt)
            nc.sync.dma_start(bk, bk_d[:].to_broadcast((T, H * HD)))
            bv = cp.tile([T, H * HD], dt)
            nc.sync.dma_start(bv, bv_d[:].to_broadcast((T, H * HD)))
            bo = cp.tile([T, D], dt)
            nc.sync.dma_start(bo, bo_d[:].to_broadcast((T, D)))
            qse = cp.tile([T, HD], dt)
            nc.sync.dma_start(qse, qse_d[:].to_broadcast((T, HD)))
            kse = cp.tile([T, HD], dt)
            nc.sync.dma_start(kse, kse_d[:].to_broadcast((T, HD)))
            onesr = cp.tile([1, H * S], dt)
            nc.sync.dma_start(onesr, onesr_d[:])
            cosq = cp.tile([T, HD], dt)
            nc.sync.dma_start(cosq, cosq_d[:])
            ssq = cp.tile([T, HD], dt)
            nc.sync.dma_start(ssq, ssq_d[:])
            cosk = cp.tile([T, H * HD], dt)
            nc.sync.dma_start(cosk, cosk_d[:])
            ssk = cp.tile([T, H * HD], dt)
            nc.sync.dma_start(ssk, ssk_d[:])
            eps_t = cp.tile([128, 1], dt)
            nc.vector.memset(eps_t, EPS)

            # ---- phase 1: projections q, k, v  (out = x @ W + b) ----
            q_pre = work.tile([T, HD], dt)
            pq = pprojp.tile([T, 512], dt, tag="pproj")
            for ch in range(16):
                wsl = wp.tile([128, 512], dt, tag="wsl")
                nc.sync.dma_start(wsl[:, 0:HD], wq_d[ch * 128:(ch + 1) * 128, :])
                nc.tensor.matmul(pq[:, 0:HD], xt[:, ch, :], wsl[:, 0:HD],
                                 start=(ch == 0), stop=(ch == 15))
            nc.vector.tensor_tensor(q_pre, pq[:, 0:HD], bq, op=addop)

            k_pre = work.tile([T, H * HD], dt)
            v_sb = work.tile([T, H * HD], dt)
            for (w_d, b_sb, dst) in ((wk_d, bk, k_pre), (wv_d, bv, v_sb)):
                for j in range(4):
                    pp = pprojp.tile([T, 512], dt, tag="pproj")
                    for ch in range(16):
                        wsl = wp.tile([128, 512], dt, tag="wsl")
                        nc.sync.dma_start(
                            wsl, w_d[ch * 128:(ch + 1) * 128, j * 512:(j + 1) * 512])
                        nc.tensor.matmul(pp, xt[:, ch, :], wsl,
                                         start=(ch == 0), stop=(ch == 15))
                    nc.vector.tensor_tensor(
                        dst[:, j * 512:(j + 1) * 512], pp,
                        b_sb[:, j * 512:(j + 1) * 512], op=addop)

            # ---- phase 2: rmsnorm + rope ----
            # q: [T, HD]
            scrq = work.tile([T, HD], dt)
            qvar = work.tile([T, 1], dt)
            qrinv = work.tile([T, 1], dt)
            nc.vector.tensor_tensor(scrq, q_pre, q_pre, op=mult)
            nc.vector.tensor_reduce(qvar, scrq, axis=mybir.AxisListType.X, op=addop)
            nc.scalar.activation(scrq[:, 0:1], qvar, sqrtf, bias=eps_t[0:T, :], scale=1.0 / HD)
            nc.vector.reciprocal(qrinv, scrq[:, 0:1])
            qn = work.tile([T, HD], dt)
            # (q_pre * rinv) * (q_scale/sqrt(HD))
            nc.vector.scalar_tensor_tensor(
                qn, q_pre, qrinv, qse, op0=mult, op1=mult)
            qrot = work.tile([T, HD], dt)
            nc.vector.tensor_copy(qrot[:, 0:64], qn[:, 64:128])
            nc.vector.tensor_copy(qrot[:, 64:128], qn[:, 0:64])
            q_rope = work.tile([T, HD], dt)
            nc.vector.tensor_tensor(scrq, qn, cosq, op=mult)
            nc.vector.tensor_tensor(qrot, qrot, ssq, op=mult)
            nc.vector.tensor_tensor(q_rope, scrq, qrot, op=addop)

            # k: [T, H*HD], per-head rmsnorm
            scr1 = work.tile([T, H * HD], dt)
            scr2 = work.tile([T, H * HD], dt)
            kvar = work.tile([T, H], dt)
            krinv = work.tile([T, H], dt)
            s1_3 = scr1.rearrange("p (h d) -> p h d", h=H)
            s2_3 = scr2.rearrange("p (h d) -> p h d", h=H)
            k3 = k_pre.rearrange("p (h d) -> p h d", h=H)
            nc.vector.tensor_tensor(scr1, k_pre, k_pre, op=mult)
            nc.vector.tensor_reduce(kvar, s1_3, axis=mybir.AxisListType.X, op=addop)
            nc.scalar.activation(scr1[:, 0:H], kvar, sqrtf, bias=eps_t[0:T, :], scale=1.0 / HD)
            nc.vector.reciprocal(krinv, scr1[:, 0:H])
            nc.vector.tensor_tensor(
                s2_3, k3, krinv.unsqueeze(2).broadcast_to((T, H, HD)), op=mult)
            nc.vector.tensor_tensor(
                s1_3, s2_3, kse.unsqueeze(1).broadcast_to((T, H, HD)),
                op=mult)  # scr1 = k normed+scaled
            # half-swap into scr2
            nc.vector.tensor_copy(s2_3[:, :, 0:64], s1_3[:, :, 64:128])
            nc.vector.tensor_copy(s2_3[:, :, 64:128], s1_3[:, :, 0:64])
            k_rope = work.tile([T, H * HD], dt)
            nc.vector.tensor_tensor(scr1, scr1, cosk, op=mult)
            nc.vector.tensor_tensor(scr2, scr2, ssk, op=mult)
            nc.vector.tensor_tensor(k_rope, scr1, scr2, op=addop)

            # write k/v outputs
            nc.sync.dma_start(ko_d[:], k_rope)
            nc.sync.dma_start(vo_d[:], v_sb)

            # ---- phase 3: transposes + per-batch small tiles ----
            qt = work.tile([128, T], dt)  # q^T [d, tok]
            pt_q = pstp.tile([128, 128], dt, tag="pst")
            nc.tensor.transpose(pt_q[:, 0:T], q_rope, ident[0:T, 0:T])
            nc.any.tensor_copy(qt, pt_q[:, 0:T])

            ktn = work.tile([128, H, T], dt)  # k_new^T [d, h, tok]
            k3r = k_rope.rearrange("p (h d) -> p h d", h=H)
            for h in range(H):
                pt_k = pstp.tile([128, 128], dt, tag="pst")
                nc.tensor.transpose(pt_k[:, 0:T], k3r[:, h, :], ident[0:T, 0:T])
                nc.any.tensor_copy(ktn[:, h, :], pt_k[:, 0:T])

            # v_new per batch at partitions 0..7 (SBUF->SBUF DMA shifts partitions)
            vbs = []
            for b in range(BPC):
                vb = work.tile([S, H, HD], dt, tag=f"vb{b}", name=f"vb{b}")
                nc.sync.dma_start(vb.rearrange("p h d -> p (h d)"),
                                  v_sb[b * S:(b + 1) * S, :])
                vbs.append(vb)

            # ---- phase 4: attention over the cache prefix + new tokens ----
            den = pdenp.tile([1, BPC * H * S], dt)  # softmax denominators
            xn = work.tile([128, H, T], dt)         # x^T normalized [d, h, tok]
            for b in range(BPC):
                x_b = pxp.tile([128, H * S], dt, tag="px", name=f"x_b{b}")
                for c in range(NCH):
                    kt = kvp.tile([128, H * HD], dt, tag="kt", name=f"kt{b}_{c}")
                    nc.sync.dma_start(kt, ck_d[b, c * 128:(c + 1) * 128, :])
                    vt = kvp.tile([128, H * HD], dt, tag="vt", name=f"vt{b}_{c}")
                    nc.sync.dma_start(vt, cv_d[b, c * 128:(c + 1) * 128, :])
                    ps_s = pssp.tile([128, H * S], dt, tag="pss",
                                     name=f"ps_s{b}_{c}")
                    for h in range(H):
                        ptr = pstp.tile([128, 128], dt, tag="pst",
                                        name=f"ptr{b}_{c}_{h}")
                        nc.tensor.transpose(ptr, kt[:, h * HD:(h + 1) * HD], ident)
                        ktr = ktrp.tile([128, 128], dt, tag="ktr",
                                        name=f"ktr{b}_{c}_{h}")
                        if h % 2 == 0:
                            nc.vector.tensor_copy(ktr, ptr)
                        else:
                            nc.scalar.copy(ktr, ptr)
                        nc.tensor.matmul(ps_s[:, h * S:(h + 1) * S], ktr,
                                         qt[:, b * S:(b + 1) * S],
                                         start=(h == 0), stop=(h == 15))
                    tst = actp.tile([128, H * S], dt, tag="tst",
                                    name=f"tst{b}_{c}")
                    nc.scalar.activation(tst, ps_s, tanh, scale=1.0 / SOFTCAP)
                    pt = actp.tile([128, H * S], dt, tag="pt", name=f"pt{b}_{c}")
                    nc.scalar.activation(pt, tst, expf, scale=SOFTCAP,
                                         bias=mb[:, b * NCH + c:b * NCH + c + 1])
                    nc.tensor.matmul(den[:, b * H * S:(b + 1) * H * S], ones, pt,
                                     start=(c == 0), stop=False)
                    for h in range(H):
                        nc.tensor.matmul(x_b[:, h * S:(h + 1) * S],
                                         vt[:, h * HD:(h + 1) * HD],
                                         pt[:, h * S:(h + 1) * S],
                                         start=(c == 0 and h == 0), stop=False)

                # new tokens (positions base..base+S-1, causal among themselves)
                psn = pssp.tile([S, H * S], dt, tag="pss", name=f"psn{b}")
                for h in range(H):
                    nc.tensor.matmul(psn[:, h * S:(h + 1) * S],
                                     ktn[:, h, b * S:(b + 1) * S],
                                     qt[:, b * S:(b + 1) * S],
                                     start=(h == 0), stop=(h == H - 1))
                tsn = actp.tile([S, H * S], dt, tag="tsn", name=f"tsn{b}")
                nc.scalar.activation(tsn, psn, tanh, scale=1.0 / SOFTCAP)
                pn = actp.tile([S, H * S], dt, tag="pn", name=f"pn{b}")
                nc.scalar.activation(pn, tsn, expf, scale=SOFTCAP)
                nc.vector.tensor_tensor(pn, pn, tri, op=mult)
                nc.tensor.matmul(den[:, b * H * S:(b + 1) * H * S], ones[0:S, :], pn,
                                 start=False, stop=True)
                for h in range(H):
                    nc.tensor.matmul(x_b[:, h * S:(h + 1) * S],
                                     vbs[b][:, h, :], pn[:, h * S:(h + 1) * S],
                                     start=False, stop=(h == H - 1))

                # normalize: xn[:, h, b*S+s] = x_b[:, (h,s)] / den
                rden = work.tile([1, H * S], dt, tag=f"rden{b}", name=f"rden{b}")
                nc.vector.reciprocal(rden, den[:, b * H * S:(b + 1) * H * S])
                rdenb = pstp.tile([128, H * S], dt, tag="pst", name=f"rdenb{b}")
                nc.tensor.matmul(rdenb, onesr, rden, start=True, stop=True)
                rdenb_sb = work.tile([128, H * S], dt, tag=f"rdenb_sb{b}",
                                     name=f"rdenb_sb{b}")
                nc.scalar.copy(rdenb_sb, rdenb)
                nc.vector.tensor_tensor(
                    xn[:, :, b * S:(b + 1) * S],
                    x_b.rearrange("p (h s) -> p h s", h=H),
                    rdenb_sb.rearrange("p (h s) -> p h s", h=H),
                    op=mult)

            # ---- phase 5: output projection ----
            out_sb = work.tile([T, D], dt)
            for j in range(4):
                po = pprojp.tile([T, 512], dt, tag="pproj", name=f"po{j}")
                for h in range(H):
                    wsl = wp.tile([128, 512], dt, tag="wsl", name=f"wo{j}_{h}")
                    nc.sync.dma_start(
                        wsl, wo_d[h * HD:(h + 1) * HD, j * 512:(j + 1) * 512])
                    nc.tensor.matmul(po, xn[:, h, :], wsl,
                                     start=(h == 0), stop=(h == 15))
                nc.vector.tensor_tensor(
                    out_sb[:, j * 512:(j + 1) * 512], po,
                    bo[:, j * 512:(j + 1) * 512], op=addop)
            nc.sync.dma_start(out_d[:], out_sb)

    return nc


def _build():
    if "nc" not in _CACHE:
        nc = bacc.Bacc("TRN2", target_bir_lowering=False, debug=False,
                       num_devices=NCORES)
        _emit(nc)
        nc.compile()
        _CACHE["nc"] = nc
    return _CACHE["nc"]


def _rope_tables(pos_flat):
    """pos_flat: [T] int -> (cos2 [T,HD], sin_signed [T,HD]) float32.

    Computed with jax on CPU to match the reference transcendentals bitwise.
    """
    import jax
    import jax.numpy as jnp

    half = HD // 2
    with jax.default_device(jax.devices("cpu")[0]):
        frac = 2.0 * jnp.arange(half, dtype=jnp.float32) / HD
        timescale = WAVELENGTH ** frac
        sinusoid = jnp.asarray(np.asarray(pos_flat, np.int32)).astype(
            jnp.float32)[:, None] / timescale[None, :]
        sin = np.asarray(jnp.sin(sinusoid), dtype=np.float32)
        cos = np.asarray(jnp.cos(sinusoid), dtype=np.float32)
    cos2 = np.concatenate([cos, cos], axis=1)
    ssg = np.concatenate([-sin, sin], axis=1)
    return np.ascontiguousarray(cos2), np.ascontiguousarray(ssg)


def _host_prep(inputs, seq_pos, cache_key, cache_value, wq, bq, wk, bk, wv, bv,
               wo, bo, q_scale, k_scale):
    """Build the 8 per-core input maps."""
    inputs = np.asarray(inputs, dtype=np.float32)
    seq_pos = np.asarray(seq_pos)
    cache_key = np.ascontiguousarray(np.asarray(cache_key, dtype=np.float32))
    cache_value = np.ascontiguousarray(np.asarray(cache_value, dtype=np.float32))
    wq2 = np.ascontiguousarray(np.asarray(wq, np.float32).reshape(D, HD))
    wk2 = np.ascontiguousarray(np.asarray(wk, np.float32).reshape(D, H * HD))
    wv2 = np.ascontiguousarray(np.asarray(wv, np.float32).reshape(D, H * HD))
    wo2 = np.ascontiguousarray(np.asarray(wo, np.float32).reshape(H * HD, D))
    bq2 = np.asarray(bq, np.float32).reshape(1, HD)
    bk2 = np.asarray(bk, np.float32).reshape(1, H * HD)
    bv2 = np.asarray(bv, np.float32).reshape(1, H * HD)
    bo2 = np.asarray(bo, np.float32).reshape(1, D)
    qse = np.ascontiguousarray(
        (np.asarray(q_scale, np.float32) /
         np.sqrt(np.float32(HD)).astype(np.float32)).reshape(1, HD))
    kse = np.ascontiguousarray(np.asarray(k_scale, np.float32).reshape(1, HD))
    ident = np.ascontiguousarray(np.eye(128, dtype=np.float32))
    ones = np.ones((128, 1), dtype=np.float32)
    # causal triangle: tri[j, h*S+s] = 1 if j <= s
    tr = (np.arange(S)[:, None] <= np.arange(S)[None, :]).astype(np.float32)
    tri = np.ascontiguousarray(np.tile(tr, (1, H)))

    in_maps = []
    for c in range(NCORES):
        b0 = c * BPC
        xc = inputs[b0:b0 + BPC].reshape(T, D)
        # xt[p, ch, t] = xc[t, ch*128+p]
        xt = np.ascontiguousarray(xc.T.reshape(16, 128, T).transpose(1, 0, 2))
        pos = seq_pos[b0:b0 + BPC].reshape(T).astype(np.int64)
        cos2, ssg = _rope_tables(pos)
        cosk = np.ascontiguousarray(np.tile(cos2, (1, H)))
        ssk = np.ascontiguousarray(np.tile(ssg, (1, H)))
        # mask bias: column b*NCH+ch, row p -> 0 if ch*128+p < base_b else MASKBIAS
        mb = np.empty((128, BPC * NCH), dtype=np.float32)
        for b in range(BPC):
            base = int(seq_pos[b0 + b, 0])
            gpos = (np.arange(NCH)[None, :] * 128 + np.arange(128)[:, None])
            mb[:, b * NCH:(b + 1) * NCH] = np.where(gpos < base, 0.0, MASKBIAS)
        in_maps.append({
            "xt": xt,
            "ck": cache_key[b0:b0 + BPC].reshape(BPC, MAXSEQ, H * HD),
            "cv": cache_value[b0:b0 + BPC].reshape(BPC, MAXSEQ, H * HD),
            "wq": wq2, "wk": wk2, "wv": wv2, "wo": wo2,
            "bq": bq2, "bk": bk2, "bv": bv2, "bo": bo2,
            "qse": qse, "kse": kse,
            "cosq": cos2, "ssq": ssg, "cosk": cosk, "ssk": ssk,
            "tri": tri, "mb": mb, "ident": ident, "ones": ones,
            "onesr": np.ones((1, H * S), dtype=np.float32),
        })
    return in_maps


def _gather(results, seq_pos, cache_key, cache_value):
    out = np.empty((B, S, D), dtype=np.float32)
    key_all = np.array(cache_key, dtype=np.float32, copy=True).reshape(
        B, MAXSEQ, H, HD)
    val_all = np.array(cache_value, dtype=np.float32, copy=True).reshape(
        B, MAXSEQ, H, HD)
    for c in range(NCORES):
        r = results[c]
        b0 = c * BPC
        out[b0:b0 + BPC] = np.asarray(r["out"]).reshape(BPC, S, D)
        k_new = np.asarray(r["ko"]).reshape(BPC, S, H, HD)
        v_new = np.asarray(r["vo"]).reshape(BPC, S, H, HD)
        for b in range(BPC):
            base = int(seq_pos[b0 + b, 0])
            key_all[b0 + b, base:base + S] = k_new[b]
            val_all[b0 + b, base:base + S] = v_new[b]
    return out, key_all, val_all


def kernel(inputs, seq_pos, cache_key, cache_value, wq, bq, wk, bk, wv, bv,
           wo, bo, q_scale, k_scale):
    nc = _build()
    in_maps = _host_prep(inputs, seq_pos, cache_key, cache_value, wq, bq, wk, bk,
                         wv, bv, wo, bo, q_scale, k_scale)
    res = bass_utils.run_bass_kernel_spmd(nc, in_maps, core_ids=list(range(NCORES)))
    return _gather(res.results, np.asarray(seq_pos), cache_key, cache_value)


# revision 11
# speedup vs baseline: 108.4368x; 108.4368x over previous
"""Trainium2 Bass kernel for a single-query-head attention block with KV-cache
update (B=16, S=8, D=2048, H=16, HD=128, MAXSEQ=4096), data-parallel over
batch across 8 NeuronCores (2 batches per core).

kernel(**inputs) takes the full unsharded numpy inputs and returns
(out, key_all, val_all) matching the reference.

Algorithm notes: seq_pos rows are contiguous (base..base+S-1), so the causal
mask over the updated cache reduces to: all original cache rows < base are
fully visible, rows >= base are replaced/masked, and the S new tokens attend
causally among themselves. The old-row mask is folded into the softmax exp()
as a per-partition additive bias (-30000 => exp underflows to 0). Masked
positions contribute < MAXSEQ*e^-50 relative to the softmax sum, far below
f32 resolution, so dropping them matches the reference within tolerance.
The updated caches are assembled on the host (8 contiguous rows per batch);
the device computes the attention output and the new k/v rows.
"""
import sys

sys.path.insert(0, "/opt/trn_rl_repo")

import numpy as np

import concourse.bacc as bacc
import concourse.bass as bass
import concourse.mybir as mybir
from concourse import tile
from concourse import bass_utils

B, S, D, H, HD, MAXSEQ = 16, 8, 2048, 16, 128, 4096
WAVELENGTH = 10000.0
SOFTCAP = 50.0
EPS = 1e-6
NCORES = 8
BPC = B // NCORES          # batches per core = 2
T = BPC * S                # tokens per core = 16
NCH = MAXSEQ // 128        # cache chunks per batch = 32
MASKBIAS = -30000.0
F32 = mybir.dt.float32

_CACHE = {}
_MODEL_NEEDS = None  # timeline-model-only skip hints


def _emit(nc):
    """Emit the per-core program. All 8 cores run this same program (SPMD)."""
    dt = F32
    # ---- DRAM I/O ----
    xt_d = nc.dram_tensor("xt", [128, 16, T], dt, kind="ExternalInput")      # x^T: [p, chunk, tok]
    ck_d = nc.dram_tensor("ck", [BPC, MAXSEQ, H * HD], dt, kind="ExternalInput")
    cv_d = nc.dram_tensor("cv", [BPC, MAXSEQ, H * HD], dt, kind="ExternalInput")
    wq_d = nc.dram_tensor("wq", [D, HD], dt, kind="ExternalInput")
    wk_d = nc.dram_tensor("wk", [D, H * HD], dt, kind="ExternalInput")
    wv_d = nc.dram_tensor("wv", [D, H * HD], dt, kind="ExternalInput")
    wo_d = nc.dram_tensor("wo", [H * HD, D], dt, kind="ExternalInput")
    bq_d = nc.dram_tensor("bq", [1, HD], dt, kind="ExternalInput")
    bk_d = nc.dram_tensor("bk", [1, H * HD], dt, kind="ExternalInput")
    bv_d = nc.dram_tensor("bv", [1, H * HD], dt, kind="ExternalInput")
    bo_d = nc.dram_tensor("bo", [1, D], dt, kind="ExternalInput")
    qse_d = nc.dram_tensor("qse", [1, HD], dt, kind="ExternalInput")         # q_scale/sqrt(HD)
    kse_d = nc.dram_tensor("kse", [1, HD], dt, kind="ExternalInput")
    cosq_d = nc.dram_tensor("cosq", [T, HD], dt, kind="ExternalInput")
    ssq_d = nc.dram_tensor("ssq", [T, HD], dt, kind="ExternalInput")         # [-sin, +sin]
    cosk_d = nc.dram_tensor("cosk", [T, H * HD], dt, kind="ExternalInput")
    ssk_d = nc.dram_tensor("ssk", [T, H * HD], dt, kind="ExternalInput")
    tri_d = nc.dram_tensor("tri", [S, H * S], dt, kind="ExternalInput")      # causal 0/1
    mb_d = nc.dram_tensor("mb", [128, BPC * NCH], dt, kind="ExternalInput")  # mask bias cols
    id_d = nc.dram_tensor("ident", [128, 128], dt, kind="ExternalInput")
    ones_d = nc.dram_tensor("ones", [128, 1], dt, kind="ExternalInput")
    onesr_d = nc.dram_tensor("onesr", [1, H * S], dt, kind="ExternalInput")
    out_d = nc.dram_tensor("out", [T, D], dt, kind="ExternalOutput")
    ko_d = nc.dram_tensor("ko", [T, H * HD], dt, kind="ExternalOutput")
    vo_d = nc.dram_tensor("vo", [T, H * HD], dt, kind="ExternalOutput")

    tanh = mybir.ActivationFunctionType.Tanh
    expf = mybir.ActivationFunctionType.Exp
    sqrtf = mybir.ActivationFunctionType.Sqrt
    mult = mybir.AluOpType.mult
    addop = mybir.AluOpType.add

    with tile.TileContext(nc) as tc:
        with (
            tc.tile_pool(name="const", bufs=1) as cp,
            tc.tile_pool(name="wp", bufs=4) as wp,
            tc.tile_pool(name="kvp", bufs=3) as kvp,
            tc.tile_pool(name="ktrp", bufs=3) as ktrp,
            tc.tile_pool(name="actp", bufs=3) as actp,
            tc.tile_pool(name="work", bufs=1) as work,
            tc.tile_pool(name="pstp", bufs=2, space="PSUM") as pstp,
            tc.tile_pool(name="pssp", bufs=2, space="PSUM") as pssp,
            tc.tile_pool(name="pprojp", bufs=2, space="PSUM") as pprojp,
            tc.tile_pool(name="pxp", bufs=1, space="PSUM") as pxp,
            tc.tile_pool(name="pdenp", bufs=1, space="PSUM") as pdenp,
        ):
            # ---- constants into SBUF ----
            ident = cp.tile([128, 128], dt)
            nc.sync.dma_start(ident, id_d[:])
            ones = cp.tile([128, 1], dt)
            nc.sync.dma_start(ones, ones_d[:])
            tri = cp.tile([S, H * S], dt)
            nc.sync.dma_start(tri, tri_d[:])
            mb = cp.tile([128, BPC * NCH], dt)
            nc.sync.dma_start(mb, mb_d[:])
            xt = cp.tile([128, 16, T], dt)
            nc.sync.dma_start(xt, xt_d[:])
            bq = cp.tile([T, HD], dt)
            nc.sync.dma_start(bq, bq_d[:].to_broadcast((T, HD)))
            bk = cp.tile([T, H * HD], dt)
            nc.sync.dma_start(bk, bk_d[:].to_broadcast((T, H * HD)))
            bv = cp.tile([T, H * HD], dt)
            nc.sync.dma_start(bv, bv_d[:].to_broadcast((T, H * HD)))
            bo = cp.tile([T, D], dt)
            nc.sync.dma_start(bo, bo_d[:].to_broadcast((T, D)))
            qse = cp.tile([T, HD], dt)
            nc.sync.dma_start(qse, qse_d[:].to_broadcast((T, HD)))
            kse = cp.tile([T, HD], dt)
            nc.sync.dma_start(kse, kse_d[:].to_broadcast((T, HD)))
            onesr = cp.tile([1, H * S], dt)
            nc.sync.dma_start(onesr, onesr_d[:])
            cosq = cp.tile([T, HD], dt)
            nc.sync.dma_start(cosq, cosq_d[:])
            ssq = cp.tile([T, HD], dt)
            nc.sync.dma_start(ssq, ssq_d[:])
            cosk = cp.tile([T, H * HD], dt)
            nc.sync.dma_start(cosk, cosk_d[:])
            ssk = cp.tile([T, H * HD], dt)
            nc.sync.dma_start(ssk, ssk_d[:])
            eps_t = cp.tile([128, 1], dt)
            nc.vector.memset(eps_t, EPS)

            # ---- phase 1: projections q, k, v  (out = x @ W + b) ----
            q_pre = work.tile([T, HD], dt)
            pq = pprojp.tile([T, 512], dt, tag="pproj")
            for ch in range(16):
                wsl = wp.tile([128, 512], dt, tag="wsl")
                nc.sync.dma_start(wsl[:, 0:HD], wq_d[ch * 128:(ch + 1) * 128, :])
                nc.tensor.matmul(pq[:, 0:HD], xt[:, ch, :], wsl[:, 0:HD],
                                 start=(ch == 0), stop=(ch == 15))
            nc.vector.tensor_tensor(q_pre, pq[:, 0:HD], bq, op=addop)

            k_pre = work.tile([T, H * HD], dt)
            v_sb = work.tile([T, H * HD], dt)
            for (w_d, b_sb, dst) in ((wk_d, bk, k_pre), (wv_d, bv, v_sb)):
                for j in range(4):
                    pp = pprojp.tile([T, 512], dt, tag="pproj")
                    for ch in range(16):
                        wsl = wp.tile([128, 512], dt, tag="wsl")
                        nc.sync.dma_start(
                            wsl, w_d[ch * 128:(ch + 1) * 128, j * 512:(j + 1) * 512])
                        nc.tensor.matmul(pp, xt[:, ch, :], wsl,
                                         start=(ch == 0), stop=(ch == 15))
                    nc.vector.tensor_tensor(
                        dst[:, j * 512:(j + 1) * 512], pp,
                        b_sb[:, j * 512:(j + 1) * 512], op=addop)

            # ---- phase 2: rmsnorm + rope ----
            # q: [T, HD]
            scrq = work.tile([T, HD], dt)
            qvar = work.tile([T, 1], dt)
            qrinv = work.tile([T, 1], dt)
            nc.vector.tensor_tensor(scrq, q_pre, q_pre, op=mult)
            nc.vector.tensor_reduce(qvar, scrq, axis=mybir.AxisListType.X, op=addop)
            nc.scalar.activation(scrq[:, 0:1], qvar, sqrtf, bias=eps_t[0:T, :], scale=1.0 / HD)
            nc.vector.reciprocal(qrinv, scrq[:, 0:1])
            qn = work.tile([T, HD], dt)
            # (q_pre * rinv) * (q_scale/sqrt(HD))
            nc.vector.scalar_tensor_tensor(
                qn, q_pre, qrinv, qse, op0=mult, op1=mult)
            qrot = work.tile([T, HD], dt)
            nc.vector.tensor_copy(qrot[:, 0:64], qn[:, 64:128])
            nc.vector.tensor_copy(qrot[:, 64:128], qn[:, 0:64])
            q_rope = work.tile([T, HD], dt)
            nc.vector.tensor_tensor(scrq, qn, cosq, op=mult)
            nc.vector.tensor_tensor(qrot, qrot, ssq, op=mult)
            nc.vector.tensor_tensor(q_rope, scrq, qrot, op=addop)

            # k: [T, H*HD], per-head rmsnorm
            scr1 = work.tile([T, H * HD], dt)
            scr2 = work.tile([T, H * HD], dt)
            kvar = work.tile([T, H], dt)
            krinv = work.tile([T, H], dt)
            s1_3 = scr1.rearrange("p (h d) -> p h d", h=H)
            s2_3 = scr2.rearrange("p (h d) -> p h d", h=H)
            k3 = k_pre.rearrange("p (h d) -> p h d", h=H)
            nc.vector.tensor_tensor(scr1, k_pre, k_pre, op=mult)
            nc.vector.tensor_reduce(kvar, s1_3, axis=mybir.AxisListType.X, op=addop)
            nc.scalar.activation(scr1[:, 0:H], kvar, sqrtf, bias=eps_t[0:T, :], scale=1.0 / HD)
            nc.vector.reciprocal(krinv, scr1[:, 0:H])
            nc.vector.tensor_tensor(
                s2_3, k3, krinv.unsqueeze(2).broadcast_to((T, H, HD)), op=mult)
            nc.vector.tensor_tensor(
                s1_3, s2_3, kse.unsqueeze(1).broadcast_to((T, H, HD)),
                op=mult)  # scr1 = k normed+scaled
            # half-swap into scr2
            nc.vector.tensor_copy(s2_3[:, :, 0:64], s1_3[:, :, 64:128])
            nc.vector.tensor_copy(s2_3[:, :, 64:128], s1_3[:, :, 0:64])
            k_rope = work.tile([T, H * HD], dt)
            nc.vector.tensor_tensor(scr1, scr1, cosk, op=mult)
            nc.vector.tensor_tensor(scr2, scr2, ssk, op=mult)
            nc.vector.tensor_tensor(k_rope, scr1, scr2, op=addop)

            # write k/v outputs
            nc.sync.dma_start(ko_d[:], k_rope)
            nc.sync.dma_start(vo_d[:], v_sb)

            # ---- phase 3: transposes + per-batch small tiles ----
            qt = work.tile([128, T], dt)  # q^T [d, tok]
            pt_q = pstp.tile([128, 128], dt, tag="pst")
            nc.tensor.transpose(pt_q[:, 0:T], q_rope, ident[0:T, 0:T])
            nc.any.tensor_copy(qt, pt_q[:, 0:T])

            ktn = work.tile([128, H, T], dt)  # k_new^T [d, h, tok]
            k3r = k_rope.rearrange("p (h d) -> p h d", h=H)
            for h in range(H):
                pt_k = pstp.tile([128, 128], dt, tag="pst")
                nc.tensor.transpose(pt_k[:, 0:T], k3r[:, h, :], ident[0:T, 0:T])
                nc.any.tensor_copy(ktn[:, h, :], pt_k[:, 0:T])

            # v_new per batch at partitions 0..7 (SBUF->SBUF DMA shifts partitions)
            vbs = []
            for b in range(BPC):
                vb = work.tile([S, H, HD], dt, tag=f"vb{b}", name=f"vb{b}")
                nc.sync.dma_start(vb.rearrange("p h d -> p (h d)"),
                                  v_sb[b * S:(b + 1) * S, :])
                vbs.append(vb)

            # ---- phase 4: attention over the cache prefix + new tokens ----
            den = pdenp.tile([1, BPC * H * S], dt)  # softmax denominators
            xn = work.tile([128, H, T], dt)         # x^T normalized [d, h, tok]
            for b in range(BPC):
                x_b = pxp.tile([128, H * S], dt, tag="px", name=f"x_b{b}")
                for c in range(NCH):
                    kt = kvp.tile([128, H * HD], dt, tag="kt", name=f"kt{b}_{c}")
                    nc.sync.dma_start(kt, ck_d[b, c * 128:(c + 1) * 128, :])
                    vt = kvp.tile([128, H * HD], dt, tag="vt", name=f"vt{b}_{c}")
                    nc.sync.dma_start(vt, cv_d[b, c * 128:(c + 1) * 128, :])
                    ps_s = pssp.tile([128, H * S], dt, tag="pss",
                                     name=f"ps_s{b}_{c}")
                    for h in range(H):
                        ptr = pstp.tile([128, 128], dt, tag="pst",
                                        name=f"ptr{b}_{c}_{h}")
                        nc.tensor.transpose(ptr, kt[:, h * HD:(h + 1) * HD], ident)
                        ktr = ktrp.tile([128, 128], dt, tag="ktr",
                                        name=f"ktr{b}_{c}_{h}")
                        if h % 2 == 0:
                            nc.vector.tensor_copy(ktr, ptr)
                        else:
                            nc.scalar.copy(ktr, ptr)
                        nc.tensor.matmul(ps_s[:, h * S:(h + 1) * S], ktr,
                                         qt[:, b * S:(b + 1) * S],
                                         start=(h == 0), stop=(h == 15))
                    tst = actp.tile([128, H * S], dt, tag="tst",
                                    name=f"tst{b}_{c}")
                    nc.scalar.activation(tst, ps_s, tanh, scale=1.0 / SOFTCAP)
                    pt = actp.tile([128, H * S], dt, tag="pt", name=f"pt{b}_{c}")
                    nc.scalar.activation(pt, tst, expf, scale=SOFTCAP,
                                         bias=mb[:, b * NCH + c:b * NCH + c + 1])
                    nc.tensor.matmul(den[:, b * H * S:(b + 1) * H * S], ones, pt,
                                     start=(c == 0), stop=False)
                    for h in range(H):
                        nc.tensor.matmul(x_b[:, h * S:(h + 1) * S],
                                         vt[:, h * HD:(h + 1) * HD],
                                         pt[:, h * S:(h + 1) * S],
                                         start=(c == 0 and h == 0), stop=False)

                # new tokens (positions base..base+S-1, causal among themselves)
                psn = pssp.tile([S, H * S], dt, tag="pss", name=f"psn{b}")
                for h in range(H):
                    nc.tensor.matmul(psn[:, h * S:(h + 1) * S],
                                     ktn[:, h, b * S:(b + 1) * S],
                                     qt[:, b * S:(b + 1) * S],
                                     start=(h == 0), stop=(h == H - 1))
                tsn = actp.tile([S, H * S], dt, tag="tsn", name=f"tsn{b}")
                nc.scalar.activation(tsn, psn, tanh, scale=1.0 / SOFTCAP)
                pn = actp.tile([S, H * S], dt, tag="pn", name=f"pn{b}")
                nc.scalar.activation(pn, tsn, expf, scale=SOFTCAP)
                nc.vector.tensor_tensor(pn, pn, tri, op=mult)
                nc.tensor.matmul(den[:, b * H * S:(b + 1) * H * S], ones[0:S, :], pn,
                                 start=False, stop=True)
                for h in range(H):
                    nc.tensor.matmul(x_b[:, h * S:(h + 1) * S],
                                     vbs[b][:, h, :], pn[:, h * S:(h + 1) * S],
                                     start=False, stop=(h == H - 1))

                # normalize: xn[:, h, b*S+s] = x_b[:, (h,s)] / den
                rden = work.tile([1, H * S], dt, tag=f"rden{b}", name=f"rden{b}")
                nc.vector.reciprocal(rden, den[:, b * H * S:(b + 1) * H * S])
                rdenb = pstp.tile([128, H * S], dt, tag="pst", name=f"rdenb{b}")
                nc.tensor.matmul(rdenb, onesr, rden, start=True, stop=True)
                rdenb_sb = work.tile([128, H * S], dt, tag=f"rdenb_sb{b}",
                                     name=f"rdenb_sb{b}")
                nc.scalar.copy(rdenb_sb, rdenb)
                nc.vector.tensor_tensor(
                    xn[:, :, b * S:(b + 1) * S],
                    x_b.rearrange("p (h s) -> p h s", h=H),
                    rdenb_sb.rearrange("p (h s) -> p h s", h=H),
                    op=mult)

            # ---- phase 5: output projection ----
            out_sb = work.tile([T, D], dt)
            for j in range(4):
                po = pprojp.tile([T, 512], dt, tag="pproj", name=f"po{j}")
                for h in range(H):
                    wsl = wp.tile([128, 512], dt, tag="wsl", name=f"wo{j}_{h}")
                    nc.sync.dma_start(
                        wsl, wo_d[h * HD:(h + 1) * HD, j * 512:(j + 1) * 512])
                    nc.tensor.matmul(po, xn[:, h, :], wsl,
                                     start=(h == 0), stop=(h == 15))
                nc.vector.tensor_tensor(
                    out_sb[:, j * 512:(j + 1) * 512], po,
                    bo[:, j * 512:(j + 1) * 512], op=addop)
            nc.sync.dma_start(out_d[:], out_sb)

    return nc


def _build():
    if "nc" not in _CACHE:
        nc = bacc.Bacc("TRN2", target_bir_lowering=False, debug=False,
                       num_devices=NCORES)
        _emit(nc)
        nc.compile()
        _CACHE["nc"] = nc
    return _CACHE["nc"]


def _rope_tables(pos_flat):
    """pos_flat: [T] int -> (cos2 [T,HD], sin_signed [T,HD]) float32.

    Computed with jax on CPU to match the reference transcendentals bitwise.
    """
    import jax
    import jax.numpy as jnp

    half = HD // 2
    with jax.default_device(jax.devices("cpu")[0]):
        frac = 2.0 * jnp.arange(half, dtype=jnp.float32) / HD
        timescale = WAVELENGTH ** frac
        sinusoid = jnp.asarray(np.asarray(pos_flat, np.int32)).astype(
            jnp.float32)[:, None] / timescale[None, :]
        sin = np.asarray(jnp.sin(sinusoid), dtype=np.float32)
        cos = np.asarray(jnp.cos(sinusoid), dtype=np.float32)
    cos2 = np.concatenate([cos, cos], axis=1)
    ssg = np.concatenate([-sin, sin], axis=1)
    return np.ascontiguousarray(cos2), np.ascontiguousarray(ssg)


def _host_prep(inputs, seq_pos, cache_key, cache_value, wq, bq, wk, bk, wv, bv,
               wo, bo, q_scale, k_scale):
    """Build the 8 per-core input maps."""
    inputs = np.asarray(inputs, dtype=np.float32)
    seq_pos = np.asarray(seq_pos)
    cache_key = np.ascontiguousarray(np.asarray(cache_key, dtype=np.float32))
    cache_value = np.ascontiguousarray(np.asarray(cache_value, dtype=np.float32))
    wq2 = np.ascontiguousarray(np.asarray(wq, np.float32).reshape(D, HD))
    wk2 = np.ascontiguousarray(np.asarray(wk, np.float32).reshape(D, H * HD))
    wv2 = np.ascontiguousarray(np.asarray(wv, np.float32).reshape(D, H * HD))
    wo2 = np.ascontiguousarray(np.asarray(wo, np.float32).reshape(H * HD, D))
    bq2 = np.asarray(bq, np.float32).reshape(1, HD)
    bk2 = np.asarray(bk, np.float32).reshape(1, H * HD)
    bv2 = np.asarray(bv, np.float32).reshape(1, H * HD)
    bo2 = np.asarray(bo, np.float32).reshape(1, D)
    qse = np.ascontiguousarray(
        (np.asarray(q_scale, np.float32) /
         np.sqrt(np.float32(HD)).astype(np.float32)).reshape(1, HD))
    kse = np.ascontiguousarray(np.asarray(k_scale, np.float32).reshape(1, HD))
    ident = np.ascontiguousarray(np.eye(128, dtype=np.float32))
    ones = np.ones((128, 1), dtype=np.float32)
    # causal triangle: tri[j, h*S+s] = 1 if j <= s
    tr = (np.arange(S)[:, None] <= np.arange(S)[None, :]).astype(np.float32)
    tri = np.ascontiguousarray(np.tile(tr, (1, H)))

    in_maps = []
    for c in range(NCORES):
        b0 = c * BPC
        xc = inputs[b0:b0 + BPC].reshape(T, D)
        # xt[p, ch, t] = xc[t, ch*128+p]
        xt = np.ascontiguousarray(xc.T.reshape(16, 128, T).transpose(1, 0, 2))
        pos = seq_pos[b0:b0 + BPC].reshape(T).astype(np.int64)
        cos2, ssg = _rope_tables(pos)
        cosk = np.ascontiguousarray(np.tile(cos2, (1, H)))
        ssk = np.ascontiguousarray(np.tile(ssg, (1, H)))
        # mask bias: column b*NCH+ch, row p -> 0 if ch*128+p < base_b else MASKBIAS
        mb = np.empty((128, BPC * NCH), dtype=np.float32)
        for b in range(BPC):
            base = int(seq_pos[b0 + b, 0])
            gpos = (np.arange(NCH)[None, :] * 128 + np.arange(128)[:, None])
            mb[:, b * NCH:(b + 1) * NCH] = np.where(gpos < base, 0.0, MASKBIAS)
        in_maps.append({
            "xt": xt,
            "ck": cache_key[b0:b0 + BPC].reshape(BPC, MAXSEQ, H * HD),
            "cv": cache_value[b0:b0 + BPC].reshape(BPC, MAXSEQ, H * HD),
            "wq": wq2, "wk": wk2, "wv": wv2, "wo": wo2,
            "bq": bq2, "bk": bk2, "bv": bv2, "bo": bo2,
            "qse": qse, "kse": kse,
            "cosq": cos2, "ssq": ssg, "cosk": cosk, "ssk": ssk,
            "tri": tri, "mb": mb, "ident": ident, "ones": ones,
            "onesr": np.ones((1, H * S), dtype=np.float32),
        })
    return in_maps


def _gather(results, seq_pos, cache_key, cache_value):
    out = np.empty((B, S, D), dtype=np.float32)
    key_all = np.array(cache_key, dtype=np.float32, copy=True).reshape(
        B, MAXSEQ, H, HD)
    val_all = np.array(cache_value, dtype=np.float32, copy=True).reshape(
        B, MAXSEQ, H, HD)
    for c in range(NCORES):
        r = results[c]
        b0 = c * BPC
        out[b0:b0 + BPC] = np.asarray(r["out"]).reshape(BPC, S, D)
        k_new = np.asarray(r["ko"]).reshape(BPC, S, H, HD)
        v_new = np.asarray(r["vo"]).reshape(BPC, S, H, HD)
        for b in range(BPC):
            base = int(seq_pos[b0 + b, 0])
            key_all[b0 + b, base:base + S] = k_new[b]
            val_all[b0 + b, base:base + S] = v_new[b]
    return out, key_all, val_all


def kernel(inputs, seq_pos, cache_key, cache_value, wq, bq, wk, bk, wv, bv,
           wo, bo, q_scale, k_scale):
    nc = _build()
    in_maps = _host_prep(inputs, seq_pos, cache_key, cache_value, wq, bq, wk, bk,
                         wv, bv, wo, bo, q_scale, k_scale)
    res = bass_utils.run_bass_kernel_spmd(nc, in_maps, core_ids=list(range(NCORES)))
    return _gather(res.results, np.asarray(seq_pos), cache_key, cache_value)
